# revision 3
# baseline (speedup 1.0000x reference)
"""CosFace loss (B=1024, D=512, C=100000) on 8 Trainium2 NeuronCores.

Strategy (tensor-parallel classification head, per sharding hint):
  - Classes sharded 12500/core (padded to 12544 = 98*128 with zero rows).
  - Host prep: x rows and weight rows L2-normalized, scaled by 16 and
    quantized to fp8 e4m3 (TRN variant, max +-240).  Quantization noise
    on each logit is ~0.1 absolute; the resulting loss error is ~1.5e-4
    relative (validated in numpy against the fp32 reference), far inside
    the 2e-2 gate.  Shards transposed to [D, C_loc] (contraction dim D
    on partitions); x transposed likewise.
  - Device per core: stream wT chunks, fp8 DoubleRow matmuls (2 MACs /
    cell / cycle, K=256 per pass, 2 passes for D=512) accumulate cosine
    tiles [128b, 2048c] in PSUM; fused ScalarE pass computes
    exp(dot/256*S - 64) with a scalar scale and accum_out giving per-row
    partial sum-of-exp (the fixed shift 64 = S bounds |logits|, so no
    running max is needed).
  - One 4KB AllReduce of the [1024] partial sums.
  - Margin fixup: the label logit must be S*(cos-M), not S*cos.  Each
    core computes cos_label from the fp32 normalized x and w[label]
    rows, swaps the two exp terms in the reduced sum, then
    logz = 64 + ln(sum), loss = mean(logz - (S*cos_label - S*M)).
    Core 0's output is used.
"""

import numpy as np
import ml_dtypes

import concourse.bass as bass
import concourse.mybir as mybir
import concourse.tile as tile
from concourse import bacc
from concourse.bass_utils import run_bass_kernel_spmd

B, D, C = 1024, 512, 100000
S, MARGIN = 64.0, 0.35
SHIFT = 64.0
QS = 16.0                     # fp8 quantization scale for both operands
NCORES = 8
CSHARD = C // NCORES          # 12500 real classes per core
CLOC = 12544                  # padded (98 * 128)
KT = D // 128                 # 4 contraction tiles of 128
KP = KT // 2                  # 2 DoubleRow passes (K=256 each)
MT = B // 128                 # 8 batch tiles
import os as _os
CHUNK = int(_os.environ.get('KCHUNK', '2048'))   # classes per PSUM group
NCHUNK = (CLOC + CHUNK - 1) // CHUNK
WBUFS = int(_os.environ.get('WBUFS', '3'))
PBUFS = int(_os.environ.get('PBUFS', '2'))

F32 = mybir.dt.float32
FP8 = mybir.dt.float8e4
AF = mybir.ActivationFunctionType
AX = mybir.AxisListType
ALU = mybir.AluOpType
DR = mybir.MatmulPerfMode.DoubleRow

_NC = None
LAST_RESULTS = None
ABLATE = _os.environ.get('ABLATE', 'full')  # full | noexp | nomm


def _body(nc, tc, xt, xn, wt, wl, loss, collective=True):
    from contextlib import ExitStack
    with ExitStack() as ctx:
        singles = ctx.enter_context(tc.tile_pool(name="singles", bufs=1))
        wpool = ctx.enter_context(tc.tile_pool(name="wpool", bufs=WBUFS))
        psump = ctx.enter_context(tc.tile_pool(name="psump", bufs=PBUFS, space="PSUM"))
        dram = ctx.enter_context(tc.tile_pool(name="dram", bufs=1, space="DRAM"))
        # ---- resident inputs ----
        xt_sb = singles.tile([128, KT, B], FP8)
        nc.sync.dma_start(out=xt_sb[:, :, :],
                          in_=xt.ap().rearrange("(k p) b -> p k b", p=128))
        xn_sb = singles.tile([128, MT, D], F32)
        nc.sync.dma_start(out=xn_sb[:, :, :],
                          in_=xn.ap().rearrange("(m p) d -> p m d", p=128))
        wl_sb = singles.tile([128, MT, D], F32)
        nc.sync.dma_start(out=wl_sb[:, :, :],
                          in_=wl.ap().rearrange("(m p) d -> p m d", p=128))

        # const bias columns (activation bias must be a [P,1] AP)
        def const_col(val):
            t = singles.tile([128, 1], F32)
            nc.vector.memset(t[:, :], val)
            return t

        cb_m64 = const_col(-SHIFT)
        cb_m86 = const_col(-(SHIFT + S * MARGIN))
        cb_p86 = const_col(SHIFT + S * MARGIN - 66.0 * float(np.log(2.0)))

        # exp argument is dot * ESCALE - 64, ESCALE folds the two fp8
        # quantization scales into S
        ESCALE = float(S / (QS * QS))

        # ---- main loop: cosine matmuls + fused exp/accumulate ----
        sums = singles.tile([128, MT, NCHUNK], F32)
        nc.vector.memset(sums[:, :, :], 0.0)
        wt_v = wt.ap().rearrange("(k p) c -> p k c", p=128)
        for c in range(NCHUNK):
            c0 = c * CHUNK
            ncls = min(CHUNK, CLOC - c0)
            wt_c = wpool.tile([128, KT, CHUNK], FP8, tag="wt")
            nc.sync.dma_start(out=wt_c[:, :, :ncls], in_=wt_v[:, :, c0:c0 + ncls])
            for m in range(MT):
                g = psump.tile([128, CHUNK], F32, tag="g")
                if ABLATE != "nomm":
                    for kp in range(KP):
                        lhsT = xt_sb[:, 2 * kp:2 * kp + 2, m * 128:(m + 1) * 128]
                        for n in range(0, ncls, 512):
                            nsz = min(512, ncls - n)
                            nc.tensor.matmul(g[:, n:n + nsz], lhsT,
                                             wt_c[:, 2 * kp:2 * kp + 2, n:n + nsz],
                                             start=(kp == 0), stop=(kp == KP - 1),
                                             perf_mode=DR)
                if ABLATE != "noexp":
                    # in-place on PSUM: we only need accum_out; ScalarE is
                    # closest to PSUM and this avoids an SBUF scratch write
                    nc.scalar.activation(g[:, :ncls], g[:, :ncls], AF.Exp,
                                         bias=cb_m64[:, :], scale=ESCALE,
                                         accum_out=sums[:, m, c:c + 1])

        # ---- label fixup (independent of main loop; overlaps) ----
        # xn/wl are pre-normalized on the host, so cos_label is a plain dot
        prod = singles.tile([128, MT, D], F32)
        nc.vector.tensor_mul(prod[:, :, :], xn_sb[:, :, :], wl_sb[:, :, :])
        cl = singles.tile([128, MT], F32)
        nc.vector.tensor_reduce(cl[:, :], prod[:, :, :], axis=AX.X, op=ALU.add)
        scl = singles.tile([128, MT], F32)
        nc.vector.tensor_scalar_mul(scl[:, :], cl[:, :], S)
        e_old = singles.tile([128, MT], F32)
        nc.scalar.activation(e_old[:, :], scl[:, :], AF.Exp, bias=cb_m64[:, :])
        e_new = singles.tile([128, MT], F32)
        nc.scalar.activation(e_new[:, :], scl[:, :], AF.Exp,
                             bias=cb_m86[:, :])
        dneg = singles.tile([128, MT], F32)
        nc.vector.tensor_sub(dneg[:, :], e_old[:, :], e_new[:, :])

        # ---- reduce partials + AllReduce ----
        se_part = singles.tile([128, MT], F32)
        nc.vector.tensor_reduce(se_part[:, :], sums[:, :, :], axis=AX.X, op=ALU.add)
        full_se = singles.tile([128, MT], F32)
        if collective:
            ar_in = dram.tile([128, MT], F32)
            ar_out = dram.tile([128, MT], F32, addr_space="Shared")
            nc.sync.dma_start(out=ar_in[:, :], in_=se_part[:, :])
            nc.gpsimd.collective_compute(
                "AllReduce", ALU.add,
                replica_groups=[list(range(NCORES))],
                ins=[ar_in.opt()], outs=[ar_out.opt()])
            nc.sync.dma_start(out=full_se[:, :], in_=ar_out[:, :])
        else:
            nc.vector.tensor_scalar_mul(full_se[:, :], se_part[:, :], float(NCORES))

        # ---- logz and loss ----
        adj = singles.tile([128, MT], F32)
        nc.vector.tensor_sub(adj[:, :], full_se[:, :], dneg[:, :])
        ln_adj = singles.tile([128, MT], F32)
        # ACT Ln is inaccurate for tiny args (~1e-21); prescale into [0.1, 10]
        # via the free affine input (ln(adj*2^66) = ln(adj) + 66*ln2, the
        # constant is folded into the final bias below).
        nc.scalar.activation(ln_adj[:, :], adj[:, :], AF.Ln, scale=float(2.0 ** 66))
        lossv = singles.tile([128, MT], F32)
        nc.vector.tensor_sub(lossv[:, :], ln_adj[:, :], scl[:, :])
        rowsum = singles.tile([128, 1], F32)
        junk2 = singles.tile([128, MT], F32)
        nc.scalar.activation(junk2[:, :], lossv[:, :], AF.Identity,
                             accum_out=rowsum[:, :])
        # partition-axis reduce on PE: [1,1] = ones.T @ rowsum
        ones_col = singles.tile([128, 1], F32)
        nc.vector.memset(ones_col[:, :], 1.0)
        fin_ps = psump.tile([1, 1], F32, tag="g")
        nc.tensor.matmul(fin_ps[:, :], ones_col[:, :], rowsum[:, :],
                         start=True, stop=True)
        fin = singles.tile([1, 1], F32)
        nc.scalar.activation(fin[:, :], fin_ps[:, :], AF.Identity,
                             scale=1.0 / B, bias=cb_p86[:1, :])
        nc.sync.dma_start(out=loss.ap()[:, :], in_=fin[:, :])


def _build(repeat=1, collective=True):
    nc = bacc.Bacc("TRN2", target_bir_lowering=False, debug=False,
                   num_devices=NCORES)
    xt = nc.dram_tensor("xt", [D, B], FP8, kind="ExternalInput")
    xn = nc.dram_tensor("xn", [B, D], F32, kind="ExternalInput")
    wt = nc.dram_tensor("wt", [D, CLOC], FP8, kind="ExternalInput")
    wl = nc.dram_tensor("wl", [B, D], F32, kind="ExternalInput")
    loss = nc.dram_tensor("loss", [1, 1], F32, kind="ExternalOutput")
    with tile.TileContext(nc) as tc:
        for _ in range(repeat):
            _body(nc, tc, xt, xn, wt, wl, loss, collective=collective)
    nc.compile()
    return nc


def _get_nc():
    global _NC
    if _NC is None:
        _NC = _build()
    return _NC


def _prep(inputs):
    x = np.asarray(inputs["input"], dtype=np.float32)
    label = np.asarray(inputs["label"]).astype(np.int64)
    w = np.asarray(inputs["weight"], dtype=np.float32)
    xnorm = np.sqrt((x * x).sum(axis=1, keepdims=True, dtype=np.float32))
    xu = x / np.maximum(xnorm, 1e-12)
    wnorm = np.sqrt((w * w).sum(axis=1, keepdims=True, dtype=np.float32))
    wn = w / np.maximum(wnorm, 1e-12)
    xt = np.ascontiguousarray((xu.T * QS)).astype(ml_dtypes.float8_e4m3)
    wl = np.ascontiguousarray(wn[label])
    in_maps = []
    for k in range(NCORES):
        shard = np.zeros((D, CLOC), dtype=ml_dtypes.float8_e4m3)
        shard[:, :CSHARD] = (wn[k * CSHARD:(k + 1) * CSHARD].T * QS).astype(
            ml_dtypes.float8_e4m3)
        in_maps.append({"xt": xt, "xn": xu, "wt": shard, "wl": wl})
    return in_maps


def kernel(**inputs):
    global LAST_RESULTS
    # this axon client build has no NTFF hook; a stray BASS_TRACE=1 in the
    # environment would crash run_bass_kernel_spmd on an optional import
    _os.environ["BASS_NEVER_TRACE"] = "1"
    nc = _get_nc()
    in_maps = _prep(inputs)
    res = run_bass_kernel_spmd(nc, in_maps, core_ids=list(range(NCORES)))
    LAST_RESULTS = res
    return np.asarray(res.results[0]["loss"][0, 0], dtype=np.float32)


# revision 15
# speedup vs baseline: 1.3546x; 1.3546x over previous
"""CosFace loss (B=1024, D=512, C=100000) on 8 Trainium2 NeuronCores.

Strategy (tensor-parallel classification head, per sharding hint):
  - Classes sharded 12500/core (padded to 12544 = 98*128 with zero rows).
  - Host prep: x rows and weight rows L2-normalized, scaled by 16 and
    quantized to fp8 e4m3 (TRN variant, max +-240).  Quantization noise
    on each logit is ~0.1 absolute; the resulting loss error is ~1.5e-4
    relative (validated in numpy against the fp32 reference), far inside
    the 2e-2 gate.  Shards transposed to [D, C_loc] (contraction dim D
    on partitions); x transposed likewise.
  - Device per core: stream wT chunks (fp8: 6.4MB vs 25.7MB f32), fp8
    DoubleRow matmuls (2 MACs/cell/cycle, K=256 per pass, 2 passes for
    D=512) accumulate cosine tiles [128b, 2048c] in PSUM; fused ScalarE
    pass computes exp(dot/256*S - 64) with a scalar scale and accum_out
    giving per-row partial sum-of-exp (the fixed shift 64 = S bounds
    |logits|, so no running max is needed).  ScalarE (~92us busy) and
    TensorE (~88us busy) are co-bottlenecks and overlap via PSUM
    double-buffering; both are near their engine roofline for this
    shape, so the exp work stays entirely on ScalarE (a DVE fast-exp2
    offload loses to the DVE pipeline-drain overhead).
  - Queue discipline: weight chunks own the sync HWDGE FIFO; the bf16
    label-fixup inputs ride the ACT HWDGE FIFO; every sem-waiting late
    DMA (AllReduce in/out, loss store) rides the GPSIMD SWDGE queue so
    it cannot head-of-line-block weight prefetch for the next dispatch.
  - Tile pools persist across repeat bodies (bufs=2 rotation) so
    back-to-back dispatches pipeline without WAR seams.
  - One 4KB AllReduce of the [1024] partial sums.
  - Margin fixup: the label logit must be S*(cos-M), not S*cos.  Each
    core computes cos_label from the fp32 normalized x and w[label]
    rows, swaps the two exp terms in the reduced sum, then
    logz = 64 + ln(sum), loss = mean(logz - (S*cos_label - S*M)).
    Core 0's output is used.
"""

import numpy as np
import ml_dtypes

import concourse.bass as bass
import concourse.mybir as mybir
import concourse.tile as tile
from concourse import bacc
from concourse.bass_utils import run_bass_kernel_spmd

B, D, C = 1024, 512, 100000
S, MARGIN = 64.0, 0.35
SHIFT = 64.0
QS = 16.0                     # fp8 quantization scale for both operands
NCORES = 8
CSHARD = C // NCORES          # 12500 real classes per core
CLOC = 12544                  # padded (98 * 128)
KT = D // 128                 # 4 contraction tiles of 128
KP = KT // 2                  # 2 DoubleRow passes (K=256 each)
MT = B // 128                 # 8 batch tiles
import os as _os
CHUNK = int(_os.environ.get('KCHUNK', '2048'))   # classes per PSUM group
NCHUNK = (CLOC + CHUNK - 1) // CHUNK
WBUFS = int(_os.environ.get('WBUFS', '2'))
PBUFS = int(_os.environ.get('PBUFS', '2'))
DSPLIT = int(_os.environ.get('DSPLIT', '0'))  # tail classes per chunk on DVE (off: DVE drain overhead makes it a loss)
FIXUP_BF16 = _os.environ.get('FIXUP_BF16', '1') == '1'
FIXUP_QUEUE = _os.environ.get('FIXUP_QUEUE', 'scalar')  # scalar | sync
COLLQ = _os.environ.get('COLLQ', 'gpsimd')  # sync | gpsimd | mixed

F32 = mybir.dt.float32
BF16 = mybir.dt.bfloat16
FP8 = mybir.dt.float8e4
AF = mybir.ActivationFunctionType
AX = mybir.AxisListType
ALU = mybir.AluOpType
DR = mybir.MatmulPerfMode.DoubleRow

_NC = None
LAST_RESULTS = None
ABLATE = _os.environ.get('ABLATE', 'full')  # full | noexp | nomm


def _body(nc, tc, pools, xt, xn, wt, wl, loss, collective=True):
    singles, wpool, psump, dram = pools
    if True:
        # ---- resident inputs ----
        xt_sb = singles.tile([128, KT, B], FP8)
        nc.sync.dma_start(out=xt_sb[:, :, :],
                          in_=xt.ap().rearrange("(k p) b -> p k b", p=128))
        # fixup inputs ride the ACT HWDGE queue so the weight-chunk DMAs on
        # the sync queue are not stuck behind them (separate FIFO), and are
        # bf16 to halve their HBM traffic; they are only needed by the
        # label-fixup pass which overlaps the main loop
        fixdt = BF16 if FIXUP_BF16 else F32
        fixq = nc.scalar if FIXUP_QUEUE == 'scalar' else nc.sync
        xn_sb = singles.tile([128, MT, D], fixdt)
        fixq.dma_start(out=xn_sb[:, :, :],
                       in_=xn.ap().rearrange("(m p) d -> p m d", p=128))
        wl_sb = singles.tile([128, MT, D], fixdt)
        fixq.dma_start(out=wl_sb[:, :, :],
                       in_=wl.ap().rearrange("(m p) d -> p m d", p=128))

        # const bias columns (activation bias must be a [P,1] AP)
        def const_col(val):
            t = singles.tile([128, 1], F32)
            nc.vector.memset(t[:, :], val)
            return t

        cb_m64 = const_col(-SHIFT)
        cb_m86 = const_col(-(SHIFT + S * MARGIN))
        cb_p86 = const_col(SHIFT + S * MARGIN - 66.0 * float(np.log(2.0)))

        # exp argument is dot * ESCALE - 64, ESCALE folds the two fp8
        # quantization scales into S
        ESCALE = float(S / (QS * QS))
        # DVE fast-exp2 constants: j = i32(dot*a + b); bitcast f32 gives
        # 2^63 * exp(dot*ESCALE - 64) * (1 +- 2%).  The +63 exponent bias
        # keeps j positive for the whole dot range (no clamp needed); the
        # 2^-63 is divided out when partials are combined.  The 0.0435
        # magic zeroes the mean multiplicative error of the linear
        # 2^frac interpolation (error budget validated in numpy: ~6e-5).
        LOG2E = 1.4426950408889634
        EXP2_A = float(ESCALE * LOG2E * (1 << 23))
        EXP2_B = float(((-SHIFT * LOG2E) + 190.0 - 0.0435) * (1 << 23))

        # ---- main loop: cosine matmuls + fused exp/accumulate ----
        sums = singles.tile([128, MT, NCHUNK], F32)
        nc.vector.memset(sums[:, :, :], 0.0)
        dve_sums = singles.tile([128, MT, NCHUNK], F32)
        nc.vector.memset(dve_sums[:, :, :], 0.0)
        scratch = (singles.tile([128, DSPLIT], mybir.dt.int32, name="scratch")
                   if DSPLIT else None)
        wt_v = wt.ap().rearrange("(k p) c -> p k c", p=128)
        for c in range(NCHUNK):
            c0 = c * CHUNK
            ncls = min(CHUNK, CLOC - c0)
            wt_c = wpool.tile([128, KT, CHUNK], FP8, tag="wt")
            nc.sync.dma_start(out=wt_c[:, :, :ncls], in_=wt_v[:, :, c0:c0 + ncls])
            for m in range(MT):
                g = psump.tile([128, CHUNK], F32, tag="g")
                if ABLATE != "nomm":
                    nkp = 1 if ABLATE == "kphalf" else KP
                    for kp in range(nkp):
                        lhsT = xt_sb[:, 2 * kp:2 * kp + 2, m * 128:(m + 1) * 128]
                        for n in range(0, ncls, 512):
                            nsz = min(512, ncls - n)
                            nc.tensor.matmul(g[:, n:n + nsz], lhsT,
                                             wt_c[:, 2 * kp:2 * kp + 2, n:n + nsz],
                                             start=(kp == 0), stop=(kp == nkp - 1),
                                             perf_mode=DR)
                if ABLATE != "noexp":
                    # in-place on PSUM: we only need accum_out; ScalarE is
                    # closest to PSUM and this avoids an SBUF scratch write
                    nexp = ncls // 2 if ABLATE == "exphalf" else ncls
                    na = nexp - DSPLIT if (DSPLIT and ncls == CHUNK
                                           and ABLATE == 'full') else nexp
                    nc.scalar.activation(g[:, :na], g[:, :na], AF.Exp,
                                         bias=cb_m64[:, :], scale=ESCALE,
                                         accum_out=sums[:, m, c:c + 1])
                    if na < nexp:
                        # tail classes on the otherwise-idle VectorE via the
                        # fast-exp2 bit trick: one fused affine+cast pass,
                        # one reduce pass over the f32-bitcast view
                        nc.vector.tensor_scalar(
                            out=scratch[:, :], in0=g[:, na:nexp],
                            scalar1=EXP2_A, scalar2=EXP2_B,
                            op0=ALU.mult, op1=ALU.add)
                        nc.vector.tensor_reduce(
                            dve_sums[:, m, c:c + 1],
                            scratch[:, :].bitcast(F32),
                            axis=AX.X, op=ALU.add)

        # ---- label fixup (independent of main loop; overlaps) ----
        # xn/wl are pre-normalized on the host, so cos_label is a plain dot
        prod = singles.tile([128, MT, D], F32)
        nc.vector.tensor_mul(prod[:, :, :], xn_sb[:, :, :], wl_sb[:, :, :])
        cl = singles.tile([128, MT], F32)
        nc.vector.tensor_reduce(cl[:, :], prod[:, :, :], axis=AX.X, op=ALU.add)
        scl = singles.tile([128, MT], F32)
        nc.vector.tensor_scalar_mul(scl[:, :], cl[:, :], S)
        e_old = singles.tile([128, MT], F32)
        nc.scalar.activation(e_old[:, :], scl[:, :], AF.Exp, bias=cb_m64[:, :])
        e_new = singles.tile([128, MT], F32)
        nc.scalar.activation(e_new[:, :], scl[:, :], AF.Exp,
                             bias=cb_m86[:, :])
        dneg = singles.tile([128, MT], F32)
        nc.vector.tensor_sub(dneg[:, :], e_old[:, :], e_new[:, :])

        # ---- reduce partials + AllReduce ----
        se_part = singles.tile([128, MT], F32)
        nc.vector.tensor_reduce(se_part[:, :], sums[:, :, :], axis=AX.X, op=ALU.add)
        if DSPLIT:
            se_dve = singles.tile([128, MT], F32)
            nc.vector.tensor_reduce(se_dve[:, :], dve_sums[:, :, :], axis=AX.X,
                                    op=ALU.add)
            # rescale the 2^63-biased DVE partials and fold into se_part
            nc.vector.tensor_scalar(out=se_dve[:, :], in0=se_dve[:, :],
                                    scalar1=float(2.0 ** -63), scalar2=None,
                                    op0=ALU.mult)
            nc.vector.tensor_add(se_part[:, :], se_part[:, :], se_dve[:, :])
        full_se = singles.tile([128, MT], F32)
        if collective:
            ar_in = dram.tile([128, MT], F32)
            ar_out = dram.tile([128, MT], F32, addr_space="Shared")
            # collective-adjacent DMAs ride the otherwise-idle GPSIMD SWDGE
            # queue: a sem-waiting DMA at the head of the sync HWDGE FIFO
            # would stall the next rep's weight prefetch behind it
            inq = nc.sync if COLLQ == 'mixed' else (
                nc.gpsimd if COLLQ == 'gpsimd' else nc.sync)
            outq = nc.sync if COLLQ == 'sync' else nc.gpsimd
            inq.dma_start(out=ar_in[:, :], in_=se_part[:, :])
            nc.gpsimd.collective_compute(
                "AllReduce", ALU.add,
                replica_groups=[list(range(NCORES))],
                ins=[ar_in.opt()], outs=[ar_out.opt()])
            outq.dma_start(out=full_se[:, :], in_=ar_out[:, :])
        else:
            nc.vector.tensor_scalar_mul(full_se[:, :], se_part[:, :], float(NCORES))

        # ---- logz and loss ----
        adj = singles.tile([128, MT], F32)
        nc.vector.tensor_sub(adj[:, :], full_se[:, :], dneg[:, :])
        ln_adj = singles.tile([128, MT], F32)
        # ACT Ln is inaccurate for tiny args (~1e-21); prescale into [0.1, 10]
        # via the free affine input (ln(adj*2^66) = ln(adj) + 66*ln2, the
        # constant is folded into the final bias below).
        nc.scalar.activation(ln_adj[:, :], adj[:, :], AF.Ln, scale=float(2.0 ** 66))
        lossv = singles.tile([128, MT], F32)
        nc.vector.tensor_sub(lossv[:, :], ln_adj[:, :], scl[:, :])
        rowsum = singles.tile([128, 1], F32)
        junk2 = singles.tile([128, MT], F32)
        nc.scalar.activation(junk2[:, :], lossv[:, :], AF.Identity,
                             accum_out=rowsum[:, :])
        # partition-axis reduce on PE: [1,1] = ones.T @ rowsum
        ones_col = singles.tile([128, 1], F32)
        nc.vector.memset(ones_col[:, :], 1.0)
        fin_ps = psump.tile([1, 1], F32, tag="g")
        nc.tensor.matmul(fin_ps[:, :], ones_col[:, :], rowsum[:, :],
                         start=True, stop=True)
        fin = singles.tile([1, 1], F32)
        nc.scalar.activation(fin[:, :], fin_ps[:, :], AF.Identity,
                             scale=1.0 / B, bias=cb_p86[:1, :])
        lq = nc.sync if COLLQ == 'sync' else nc.gpsimd
        lq.dma_start(out=loss.ap()[:, :], in_=fin[:, :])


def _build(repeat=1, collective=None):
    if collective is None:
        collective = _os.environ.get('NOCOLL') != '1'
    nc = bacc.Bacc("TRN2", target_bir_lowering=False, debug=False,
                   num_devices=NCORES)
    xt = nc.dram_tensor("xt", [D, B], FP8, kind="ExternalInput")
    xn = nc.dram_tensor("xn", [B, D], BF16 if FIXUP_BF16 else F32, kind="ExternalInput")
    wt = nc.dram_tensor("wt", [D, CLOC], FP8, kind="ExternalInput")
    wl = nc.dram_tensor("wl", [B, D], BF16 if FIXUP_BF16 else F32, kind="ExternalInput")
    loss = nc.dram_tensor("loss", [1, 1], F32, kind="ExternalOutput")
    from contextlib import ExitStack
    with tile.TileContext(nc) as tc, ExitStack() as ctx:
        # pools persist across reps (bufs>=2 rotation) so rep i+1's input
        # DMAs can prefetch while rep i drains -- removes the WAR seam at
        # rep boundaries; harmless for a single dispatch
        singles = ctx.enter_context(tc.tile_pool(name="singles", bufs=2))
        wpool = ctx.enter_context(tc.tile_pool(name="wpool", bufs=WBUFS))
        psump = ctx.enter_context(tc.tile_pool(name="psump", bufs=PBUFS, space="PSUM"))
        dram = ctx.enter_context(tc.tile_pool(name="dram", bufs=2, space="DRAM"))
        pools = (singles, wpool, psump, dram)
        for _ in range(repeat):
            _body(nc, tc, pools, xt, xn, wt, wl, loss, collective=collective)
    nc.compile()
    return nc


def _get_nc():
    global _NC
    if _NC is None:
        _NC = _build()
    return _NC


def _prep(inputs):
    x = np.asarray(inputs["input"], dtype=np.float32)
    label = np.asarray(inputs["label"]).astype(np.int64)
    w = np.asarray(inputs["weight"], dtype=np.float32)
    xnorm = np.sqrt((x * x).sum(axis=1, keepdims=True, dtype=np.float32))
    xu = x / np.maximum(xnorm, 1e-12)
    wnorm = np.sqrt((w * w).sum(axis=1, keepdims=True, dtype=np.float32))
    wn = w / np.maximum(wnorm, 1e-12)
    xt = np.ascontiguousarray((xu.T * QS)).astype(ml_dtypes.float8_e4m3)
    fdt = ml_dtypes.bfloat16 if FIXUP_BF16 else np.float32
    xu_bf = xu.astype(fdt)
    wl_bf = np.ascontiguousarray(wn[label]).astype(fdt)
    in_maps = []
    for k in range(NCORES):
        shard = np.zeros((D, CLOC), dtype=ml_dtypes.float8_e4m3)
        shard[:, :CSHARD] = (wn[k * CSHARD:(k + 1) * CSHARD].T * QS).astype(
            ml_dtypes.float8_e4m3)
        in_maps.append({"xt": xt, "xn": xu_bf, "wt": shard, "wl": wl_bf})
    return in_maps


def kernel(**inputs):
    global LAST_RESULTS
    # this axon client build has no NTFF hook; a stray BASS_TRACE=1 in the
    # environment would crash run_bass_kernel_spmd on an optional import
    _os.environ["BASS_NEVER_TRACE"] = "1"
    nc = _get_nc()
    in_maps = _prep(inputs)
    res = run_bass_kernel_spmd(nc, in_maps, core_ids=list(range(NCORES)))
    LAST_RESULTS = res
    return np.asarray(res.results[0]["loss"][0, 0], dtype=np.float32)


# revision 19
# speedup vs baseline: 1.7466x; 1.2894x over previous
"""CosFace loss (B=1024, D=512, C=100000) on 8 Trainium2 NeuronCores.

Strategy (tensor-parallel classification head, per sharding hint):
  - Classes sharded 12500/core (padded to 12544 = 98*128 with zero rows).
  - Host prep: x rows and weight rows L2-normalized, scaled by 16 and
    quantized to fp8 e4m3 (TRN variant, max +-240).  Quantization noise
    on each logit is ~0.1 absolute; the resulting loss error is ~1.5e-4
    relative (validated in numpy against the fp32 reference), far inside
    the 2e-2 gate.  Shards transposed to [D, C_loc] (contraction dim D
    on partitions); x transposed likewise.
  - Device per core: stream wT chunks (fp8: 6.4MB vs 25.7MB f32), fp8
    DoubleRow matmuls (2 MACs/cell/cycle, K=256 per pass, 2 passes for
    D=512) accumulate cosine tiles [128b, 2048c] in PSUM; fused ScalarE
    pass computes exp(dot/256*S - 64) with a scalar scale and accum_out
    giving per-row partial sum-of-exp (the fixed shift 64 = S bounds
    |logits|, so no running max is needed).  ScalarE (~92us busy) and
    TensorE (~88us busy) are co-bottlenecks and overlap via PSUM
    double-buffering; both are near their engine roofline for this
    shape, so the exp work stays entirely on ScalarE (a DVE fast-exp2
    offload loses to the DVE pipeline-drain overhead).
  - Queue discipline: weight chunks own the sync HWDGE FIFO; the bf16
    label-fixup inputs ride the ACT HWDGE FIFO; every sem-waiting late
    DMA (AllReduce in/out, loss store) rides the GPSIMD SWDGE queue so
    it cannot head-of-line-block weight prefetch for the next dispatch.
  - Tile pools persist across repeat bodies (bufs=2 rotation) so
    back-to-back dispatches pipeline without WAR seams.
  - One 4KB AllReduce of the [1024] partial sums.
  - Margin fixup: the label logit must be S*(cos-M), not S*cos.  Each
    core computes cos_label from the fp32 normalized x and w[label]
    rows, swaps the two exp terms in the reduced sum, then
    logz = 64 + ln(sum), loss = mean(logz - (S*cos_label - S*M)).
    Core 0's output is used.
"""

import numpy as np
import ml_dtypes

import concourse.bass as bass
import concourse.mybir as mybir
import concourse.tile as tile
from concourse import bacc
from concourse.bass_utils import run_bass_kernel_spmd

B, D, C = 1024, 512, 100000
S, MARGIN = 64.0, 0.35
SHIFT = 64.0
QS = 16.0                     # fp8 quantization scale for both operands
NCORES = 8
CSHARD = C // NCORES          # 12500 real classes per core
CLOC = 12544                  # padded (98 * 128)
KT = D // 128                 # 4 contraction tiles of 128
KP = KT // 2                  # 2 DoubleRow passes (K=256 each)
MT = B // 128                 # 8 batch tiles
import os as _os
CHUNK = int(_os.environ.get('COSFACE_KCHUNK', '1536'))   # classes per PSUM group: 2x6KB bufs + 2KB fin bank; A/B'd 81.7us vs 113.2us at 2048
NCHUNK = (CLOC + CHUNK - 1) // CHUNK
WBUFS = int(_os.environ.get('COSFACE_WBUFS', '2'))
PBUFS = int(_os.environ.get('COSFACE_PBUFS', '2'))
DSPLIT = int(_os.environ.get('COSFACE_DSPLIT', '0'))  # tail classes per chunk on DVE (off: DVE drain overhead makes it a loss)
FIXUP_BF16 = _os.environ.get('COSFACE_FIXUP_BF16', '1') == '1'
FIXUP_QUEUE = _os.environ.get('COSFACE_FIXUP_QUEUE', 'scalar')  # scalar | sync
COLLQ = _os.environ.get('COSFACE_COLLQ', 'gpsimd')  # sync | gpsimd | mixed

F32 = mybir.dt.float32
BF16 = mybir.dt.bfloat16
FP8 = mybir.dt.float8e4
AF = mybir.ActivationFunctionType
AX = mybir.AxisListType
ALU = mybir.AluOpType
DR = mybir.MatmulPerfMode.DoubleRow

_NC = None
LAST_RESULTS = None
ABLATE = _os.environ.get('COSFACE_ABLATE', 'full')  # full | noexp | nomm


def _body(nc, tc, pools, xt, xn, wt, wl, loss, collective=True):
    singles, wpool, psump, dram = pools
    if True:
        # ---- resident inputs ----
        xt_sb = singles.tile([128, KT, B], FP8)
        nc.sync.dma_start(out=xt_sb[:, :, :],
                          in_=xt.ap().rearrange("(k p) b -> p k b", p=128))
        # fixup inputs ride the ACT HWDGE queue so the weight-chunk DMAs on
        # the sync queue are not stuck behind them (separate FIFO), and are
        # bf16 to halve their HBM traffic; they are only needed by the
        # label-fixup pass which overlaps the main loop
        fixdt = BF16 if FIXUP_BF16 else F32
        fixq = nc.scalar if FIXUP_QUEUE == 'scalar' else nc.sync
        xn_sb = singles.tile([128, MT, D], fixdt)
        fixq.dma_start(out=xn_sb[:, :, :],
                       in_=xn.ap().rearrange("(m p) d -> p m d", p=128))
        wl_sb = singles.tile([128, MT, D], fixdt)
        fixq.dma_start(out=wl_sb[:, :, :],
                       in_=wl.ap().rearrange("(m p) d -> p m d", p=128))

        # const bias columns (activation bias must be a [P,1] AP)
        def const_col(val):
            t = singles.tile([128, 1], F32)
            nc.vector.memset(t[:, :], val)
            return t

        cb_m64 = const_col(-SHIFT)
        cb_m86 = const_col(-(SHIFT + S * MARGIN))
        cb_p86 = const_col(SHIFT + S * MARGIN - 66.0 * float(np.log(2.0)))

        # exp argument is dot * ESCALE - 64, ESCALE folds the two fp8
        # quantization scales into S
        ESCALE = float(S / (QS * QS))
        # DVE fast-exp2 constants: j = i32(dot*a + b); bitcast f32 gives
        # 2^63 * exp(dot*ESCALE - 64) * (1 +- 2%).  The +63 exponent bias
        # keeps j positive for the whole dot range (no clamp needed); the
        # 2^-63 is divided out when partials are combined.  The 0.0435
        # magic zeroes the mean multiplicative error of the linear
        # 2^frac interpolation (error budget validated in numpy: ~6e-5).
        LOG2E = 1.4426950408889634
        EXP2_A = float(ESCALE * LOG2E * (1 << 23))
        EXP2_B = float(((-SHIFT * LOG2E) + 190.0 - 0.0435) * (1 << 23))

        # ---- main loop: cosine matmuls + fused exp/accumulate ----
        sums = singles.tile([128, MT, NCHUNK], F32)
        nc.vector.memset(sums[:, :, :], 0.0)
        dve_sums = singles.tile([128, MT, NCHUNK], F32)
        nc.vector.memset(dve_sums[:, :, :], 0.0)
        scratch = (singles.tile([128, DSPLIT], mybir.dt.int32, name="scratch")
                   if DSPLIT else None)
        wt_v = wt.ap().rearrange("(k p) c -> p k c", p=128)
        for c in range(NCHUNK):
            c0 = c * CHUNK
            ncls = min(CHUNK, CLOC - c0)
            wt_c = wpool.tile([128, KT, CHUNK], FP8, tag="wt")
            nc.sync.dma_start(out=wt_c[:, :, :ncls], in_=wt_v[:, :, c0:c0 + ncls])
            for m in range(MT):
                g = psump.tile([128, CHUNK], F32, tag="g")
                if ABLATE != "nomm":
                    nkp = 1 if ABLATE == "kphalf" else KP
                    for kp in range(nkp):
                        lhsT = xt_sb[:, 2 * kp:2 * kp + 2, m * 128:(m + 1) * 128]
                        for n in range(0, ncls, 512):
                            nsz = min(512, ncls - n)
                            nc.tensor.matmul(g[:, n:n + nsz], lhsT,
                                             wt_c[:, 2 * kp:2 * kp + 2, n:n + nsz],
                                             start=(kp == 0), stop=(kp == nkp - 1),
                                             perf_mode=DR)
                if ABLATE != "noexp":
                    # in-place on PSUM: we only need accum_out; ScalarE is
                    # closest to PSUM and this avoids an SBUF scratch write
                    nexp = ncls // 2 if ABLATE == "exphalf" else ncls
                    na = nexp - DSPLIT if (DSPLIT and ncls == CHUNK
                                           and ABLATE == 'full') else nexp
                    nc.scalar.activation(g[:, :na], g[:, :na], AF.Exp,
                                         bias=cb_m64[:, :], scale=ESCALE,
                                         accum_out=sums[:, m, c:c + 1])
                    if na < nexp:
                        # tail classes on the otherwise-idle VectorE via the
                        # fast-exp2 bit trick: one fused affine+cast pass,
                        # one reduce pass over the f32-bitcast view
                        nc.vector.tensor_scalar(
                            out=scratch[:, :], in0=g[:, na:nexp],
                            scalar1=EXP2_A, scalar2=EXP2_B,
                            op0=ALU.mult, op1=ALU.add)
                        nc.vector.tensor_reduce(
                            dve_sums[:, m, c:c + 1],
                            scratch[:, :].bitcast(F32),
                            axis=AX.X, op=ALU.add)

        # ---- label fixup (independent of main loop; overlaps) ----
        # xn/wl are pre-normalized on the host, so cos_label is a plain dot
        prod = singles.tile([128, MT, D], F32)
        nc.vector.tensor_mul(prod[:, :, :], xn_sb[:, :, :], wl_sb[:, :, :])
        cl = singles.tile([128, MT], F32)
        nc.vector.tensor_reduce(cl[:, :], prod[:, :, :], axis=AX.X, op=ALU.add)
        scl = singles.tile([128, MT], F32)
        nc.vector.tensor_scalar_mul(scl[:, :], cl[:, :], S)
        e_old = singles.tile([128, MT], F32)
        nc.scalar.activation(e_old[:, :], scl[:, :], AF.Exp, bias=cb_m64[:, :])
        e_new = singles.tile([128, MT], F32)
        nc.scalar.activation(e_new[:, :], scl[:, :], AF.Exp,
                             bias=cb_m86[:, :])
        dneg = singles.tile([128, MT], F32)
        nc.vector.tensor_sub(dneg[:, :], e_old[:, :], e_new[:, :])

        # ---- reduce partials + AllReduce ----
        se_part = singles.tile([128, MT], F32)
        nc.vector.tensor_reduce(se_part[:, :], sums[:, :, :], axis=AX.X, op=ALU.add)
        if DSPLIT:
            se_dve = singles.tile([128, MT], F32)
            nc.vector.tensor_reduce(se_dve[:, :], dve_sums[:, :, :], axis=AX.X,
                                    op=ALU.add)
            # rescale the 2^63-biased DVE partials and fold into se_part
            nc.vector.tensor_scalar(out=se_dve[:, :], in0=se_dve[:, :],
                                    scalar1=float(2.0 ** -63), scalar2=None,
                                    op0=ALU.mult)
            nc.vector.tensor_add(se_part[:, :], se_part[:, :], se_dve[:, :])
        full_se = singles.tile([128, MT], F32)
        if collective:
            ar_in = dram.tile([128, MT], F32)
            ar_out = dram.tile([128, MT], F32, addr_space="Shared")
            # collective-adjacent DMAs ride the otherwise-idle GPSIMD SWDGE
            # queue: a sem-waiting DMA at the head of the sync HWDGE FIFO
            # would stall the next rep's weight prefetch behind it
            inq = nc.sync if COLLQ == 'mixed' else (
                nc.gpsimd if COLLQ == 'gpsimd' else nc.sync)
            outq = nc.sync if COLLQ == 'sync' else nc.gpsimd
            inq.dma_start(out=ar_in[:, :], in_=se_part[:, :])
            nc.gpsimd.collective_compute(
                "AllReduce", ALU.add,
                replica_groups=[list(range(NCORES))],
                ins=[ar_in.opt()], outs=[ar_out.opt()])
            outq.dma_start(out=full_se[:, :], in_=ar_out[:, :])
        else:
            nc.vector.tensor_scalar_mul(full_se[:, :], se_part[:, :], float(NCORES))

        # ---- logz and loss ----
        adj = singles.tile([128, MT], F32)
        nc.vector.tensor_sub(adj[:, :], full_se[:, :], dneg[:, :])
        ln_adj = singles.tile([128, MT], F32)
        # ACT Ln is inaccurate for tiny args (~1e-21); prescale into [0.1, 10]
        # via the free affine input (ln(adj*2^66) = ln(adj) + 66*ln2, the
        # constant is folded into the final bias below).
        nc.scalar.activation(ln_adj[:, :], adj[:, :], AF.Ln, scale=float(2.0 ** 66))
        lossv = singles.tile([128, MT], F32)
        nc.vector.tensor_sub(lossv[:, :], ln_adj[:, :], scl[:, :])
        rowsum = singles.tile([128, 1], F32)
        junk2 = singles.tile([128, MT], F32)
        nc.scalar.activation(junk2[:, :], lossv[:, :], AF.Identity,
                             accum_out=rowsum[:, :])
        # partition-axis reduce on PE: [1,1] = ones.T @ rowsum
        ones_col = singles.tile([128, 1], F32)
        nc.vector.memset(ones_col[:, :], 1.0)
        # own tag when PSUM has room: sharing the "g" rotation bucket makes
        # the next rep's PSUM tiles alias fin_ps and stall matmuls behind
        # this rep's collective+ln tail
        if CHUNK <= 1536:
            fin_ps = psump.tile([1, 1], F32, tag="fin", bufs=1)
        else:
            fin_ps = psump.tile([1, 1], F32, tag="g")
        nc.tensor.matmul(fin_ps[:, :], ones_col[:, :], rowsum[:, :],
                         start=True, stop=True)
        fin = singles.tile([1, 1], F32)
        nc.scalar.activation(fin[:, :], fin_ps[:, :], AF.Identity,
                             scale=1.0 / B, bias=cb_p86[:1, :])
        lq = nc.sync if COLLQ == 'sync' else nc.gpsimd
        lq.dma_start(out=loss.ap()[:, :], in_=fin[:, :])


def _build(repeat=1, collective=None):
    if collective is None:
        collective = _os.environ.get('COSFACE_NOCOLL') != '1'
    nc = bacc.Bacc("TRN2", target_bir_lowering=False, debug=False,
                   num_devices=NCORES)
    xt = nc.dram_tensor("xt", [D, B], FP8, kind="ExternalInput")
    xn = nc.dram_tensor("xn", [B, D], BF16 if FIXUP_BF16 else F32, kind="ExternalInput")
    wt = nc.dram_tensor("wt", [D, CLOC], FP8, kind="ExternalInput")
    wl = nc.dram_tensor("wl", [B, D], BF16 if FIXUP_BF16 else F32, kind="ExternalInput")
    loss = nc.dram_tensor("loss", [1, 1], F32, kind="ExternalOutput")
    from contextlib import ExitStack
    with tile.TileContext(nc) as tc, ExitStack() as ctx:
        # pools persist across reps (bufs>=2 rotation) so rep i+1's input
        # DMAs can prefetch while rep i drains -- removes the WAR seam at
        # rep boundaries; harmless for a single dispatch
        singles = ctx.enter_context(tc.tile_pool(name="singles", bufs=2))
        wpool = ctx.enter_context(tc.tile_pool(name="wpool", bufs=WBUFS))
        psump = ctx.enter_context(tc.tile_pool(name="psump", bufs=PBUFS, space="PSUM"))
        dram = ctx.enter_context(tc.tile_pool(name="dram", bufs=2, space="DRAM"))
        pools = (singles, wpool, psump, dram)
        for _ in range(repeat):
            _body(nc, tc, pools, xt, xn, wt, wl, loss, collective=collective)
    nc.compile()
    return nc


def _get_nc():
    global _NC
    if _NC is None:
        _NC = _build()
    return _NC


def _prep(inputs):
    x = np.asarray(inputs["input"], dtype=np.float32)
    label = np.asarray(inputs["label"]).astype(np.int64)
    w = np.asarray(inputs["weight"], dtype=np.float32)
    xnorm = np.sqrt((x * x).sum(axis=1, keepdims=True, dtype=np.float32))
    xu = x / np.maximum(xnorm, 1e-12)
    wnorm = np.sqrt((w * w).sum(axis=1, keepdims=True, dtype=np.float32))
    wn = w / np.maximum(wnorm, 1e-12)
    xt = np.ascontiguousarray((xu.T * QS)).astype(ml_dtypes.float8_e4m3)
    fdt = ml_dtypes.bfloat16 if FIXUP_BF16 else np.float32
    xu_bf = xu.astype(fdt)
    wl_bf = np.ascontiguousarray(wn[label]).astype(fdt)
    in_maps = []
    for k in range(NCORES):
        shard = np.zeros((D, CLOC), dtype=ml_dtypes.float8_e4m3)
        shard[:, :CSHARD] = (wn[k * CSHARD:(k + 1) * CSHARD].T * QS).astype(
            ml_dtypes.float8_e4m3)
        in_maps.append({"xt": xt, "xn": xu_bf, "wt": shard, "wl": wl_bf})
    return in_maps


def kernel(**inputs):
    global LAST_RESULTS
    # this axon client build has no NTFF hook; a stray BASS_TRACE=1 in the
    # environment would crash run_bass_kernel_spmd on an optional import
    _os.environ["BASS_NEVER_TRACE"] = "1"
    nc = _get_nc()
    in_maps = _prep(inputs)
    res = run_bass_kernel_spmd(nc, in_maps, core_ids=list(range(NCORES)))
    LAST_RESULTS = res
    return np.asarray(res.results[0]["loss"][0, 0], dtype=np.float32)
